# revision 22
# baseline (speedup 1.0000x reference)
"""Trainium2 Bass kernel for nn_EqStftPBC (STFT perturbation-based compensation).

Restructured vs baseline:
- conj-pair sharing: C computed for 11 cols/core; negative-n2 js reuse the
  partner's C via host-permuted sign-baked U weights + negative-stride APs.
- R evicted once as 3 planes [RiN|Rr|Ri] j-major; C and V products are pure
  stride-tricks over it (no extra negates).
- V products read U directly from PSUM (vector); G = 3 passes/chunk with
  {Gr, Gi, -Gi} weights (no ViN plane), single accumulating D bank.
- software-pipelined issue order so PE/vector/gpsimd/scalar overlap.

8 cores = (b x m x h), uniform SPMD program; per-core variation only in data.
"""

import numpy as np
from ml_dtypes import bfloat16

import concourse.bass as bass
import concourse.bacc as bacc
import concourse.mybir as mybir
import concourse.tile as tile

F = 80
T = 51
TP = 52
HOP = 40
L = 2080
NJ = 20          # js per core
NCOL = 11        # C columns per core
FP32 = mybir.dt.float32
BF16 = mybir.dt.bfloat16

# per-core j lists (n2 values), V-chunk order [VC0|VC1|VC2|VC3], 5 each.
# VC0/VC1 plain (C cols 0..9); VC2 conj (partner cols 0..4);
# VC3 slots 0..3 conj (partner cols 5..8), slot 4 plain (C col 10).
J_LISTS = {
    0: [1, 2, 3, 4, 5, -1, -2, -3, -4, -5,
        6, 7, 8, 9, 10, -6, -7, -8, -9, -10],
    1: [11, 12, 13, 14, 15, -11, -12, -13, -14, -15,
        16, 17, 18, 19, 0, -16, -17, -18, -19, -20],
}
# C col per slot; half-A = slots 0-9 (cols 0-4), half-B = slots 10-19 (cols 5-10)
COL_OF_SLOT = [0, 1, 2, 3, 4] * 2 + [5, 6, 7, 8, 9, 5, 6, 7, 8, 10]
CONJ_SLOT = ([False] * 5 + [True] * 5) * 2
CONJ_SLOT[19] = False   # slot 19 is plain (its C col = 10, from R slot 19)

BLC = NCOL * TP          # C plane-block width (572)
RJW = 3 * T              # Rs per-j width (153): [RiN|Rr|Ri]
QJW = 4 * T              # QV per-j width (204): [p1|p2|p3N|p4]
VBL = 10 * T             # Vt plane-block width per group (510)


def build_program(debug=False):
    nc = bacc.Bacc("TRN2", target_bir_lowering=False, debug=debug)

    xf = nc.dram_tensor("xf", [F, 3 * T], BF16, kind="ExternalInput")
    fr_c = nc.dram_tensor("fr_c", [F, 2 * F], BF16, kind="ExternalInput")
    gr_c = nc.dram_tensor("gr_c", [F, 3 * F + 2 * HOP], BF16, kind="ExternalInput")
    smat = nc.dram_tensor("smat", [F, NJ * F], BF16, kind="ExternalInput")
    mst = nc.dram_tensor("mst", [F, NJ * 3 * F], BF16, kind="ExternalInput")
    yv = nc.dram_tensor("yv", [HOP, 2 * 52], FP32, kind="ExternalOutput")

    TTv = nc.vector.tensor_tensor
    TTg = nc.gpsimd.tensor_tensor
    MUL = mybir.AluOpType.mult
    ADD = mybir.AluOpType.add
    SUB = mybir.AluOpType.subtract
    CPY = mybir.ActivationFunctionType.Copy
    ACT = nc.scalar.activation

    with tile.TileContext(nc) as tc:
        with (
            tc.tile_pool(name="const", bufs=1) as cpool,
            tc.tile_pool(name="work", bufs=1) as wpool,
            tc.tile_pool(name="pp", bufs=6, space="PSUM") as pp,
            tc.tile_pool(name="pd", bufs=1, space="PSUM") as pd,
        ):
            # ---------------- input DMAs ----------------
            frm = wpool.tile([F, 3 * T], BF16, tag="frm")
            nc.sync.dma_start(frm[:, :], xf[:, :])
            Fc = cpool.tile([F, 2 * F], BF16, tag="Fc")
            nc.sync.dma_start(Fc[:, :], fr_c[:, :])
            Ssb = cpool.tile([F, NJ * F], BF16, tag="Ssb")
            for q in range(2):
                nc.sync.dma_start(Ssb[:, q * 10 * F:(q + 1) * 10 * F],
                                  smat[:, q * 10 * F:(q + 1) * 10 * F])
            Msb = cpool.tile([F, NJ * 3 * F], BF16, tag="Msb")
            for q in range(4):
                nc.gpsimd.dma_start(Msb[:, q * 15 * F:(q + 1) * 15 * F],
                                    mst[:, q * 15 * F:(q + 1) * 15 * F])
            Gc = cpool.tile([F, 3 * F + 2 * HOP], BF16, tag="Gc")
            nc.gpsimd.dma_start(Gc[:, :], gr_c[:, :])

            # ---------------- STFT ----------------
            Xp = pp.tile([F, 2 * T], FP32, tag="ps", name="Xp")
            nc.tensor.matmul(Xp[:, :], Fc[:, 0:F], frm[:, T:3 * T], start=True, stop=False)
            nc.tensor.matmul(Xp[:, :], Fc[:, F:2 * F], frm[:, 0:2 * T], start=False, stop=True)
            Xsb = wpool.tile([F, 2 * TP], BF16, tag="Xsb")
            ACT(Xsb[:, :].rearrange("p (c t) -> p c t", c=2)[:, :, 0:T],
                Xp[:, :].rearrange("p (c t) -> p c t", c=2), CPY)
            Xrhs = bass.AP(tensor=Xsb[:, :].tensor, offset=Xsb[:, :].offset,
                           ap=[[2 * TP, F], [TP, 2], [1, T]])

            # ---------------- R matmuls + evictions ----------------
            # RB groups of 10 js -> [F, 1020] fp32 (2 banks), layout [j, c2, t]
            Rs = wpool.tile([F, NJ * RJW], BF16, tag="Rs")   # [j, (RiN|Rr|Ri), t]

            RPOS = {2: 0, 3: 1, 0: 2, 1: 3}   # smat packing position per group

            def r_group(g):
                RB = pp.tile([F, 5 * 2 * T], FP32, tag="ps", name=f"RB{g}")
                for s in range(5):
                    w = RPOS[g] * 5 + s
                    nc.tensor.matmul(RB[:, s * 2 * T:(s + 1) * 2 * T],
                                     Ssb[:, w * F:(w + 1) * F], Xrhs,
                                     start=True, stop=True)
                return RB

            def r_evictA(g, RB):
                # evict A: Rs[j, {Rr,Ri}, t] <- RB[j, c2, t]
                dstA = bass.AP(tensor=Rs[:, :].tensor,
                               offset=Rs[:, :].offset + g * 5 * RJW + T,
                               ap=[[NJ * RJW, F], [RJW, 5], [T, 2], [1, T]])
                ACT(dstA, RB[:, :].rearrange("p (j c t) -> p j c t", j=5, c=2), CPY)

            def r_evictB(g, RB):
                # evict B: Rs[j, RiN, t] <- -RB[j, 1, t]
                dstB = bass.AP(tensor=Rs[:, :].tensor,
                               offset=Rs[:, :].offset + g * 5 * RJW,
                               ap=[[NJ * RJW, F], [RJW, 5], [1, T]])
                srcB = bass.AP(tensor=RB[:, :].tensor, offset=RB[:, :].offset + T,
                               ap=[[5 * 2 * T, F], [2 * T, 5], [1, T]])
                ACT(dstB, srcB, CPY, scale=-1.0)

            RBs = {}
            for g in (2, 3, 0, 1):
                RBs[g] = r_group(g)
            for g in (2, 3, 0, 1):
                r_evictA(g, RBs[g])

            # ---------------- C stage ----------------
            # QA/QB: [F, 11, 2, 51] products; Cp: [F, 11, 52] slotted; C: 3 blocks
            QA = wpool.tile([F, NCOL * 2 * T], BF16, tag="QA")
            QB = wpool.tile([F, NCOL * 2 * T], BF16, tag="QB")
            Cp = wpool.tile([F, 2 * NCOL * TP], BF16, tag="Cp")   # [Cpr | Cpi]
            Csb = wpool.tile([F, 2 * BLC], BF16, tag="Csb")       # [Cr | Ci]

            def xbc(swap, nj):
                # [Xr|Xi] (swap=False) or [Xi|Xr] per j, broadcast over j
                off = Xsb[:, :].offset + (TP if swap else 0)
                st = -TP if swap else TP
                return bass.AP(tensor=Xsb[:, :].tensor, offset=off,
                               ap=[[2 * TP, F], [0, nj], [st, 2], [1, T]])

            def rs_rri(j0, nj):
                return bass.AP(tensor=Rs[:, :].tensor,
                               offset=Rs[:, :].offset + j0 * RJW + T,
                               ap=[[NJ * RJW, F], [RJW, nj], [T, 2], [1, T]])

            def q_view(Q, c0, nj):
                return bass.AP(tensor=Q[:, :].tensor,
                               offset=Q[:, :].offset + c0 * 2 * T,
                               ap=[[NCOL * 2 * T, F], [2 * T, nj], [T, 2], [1, T]])

            def cp_addsub(Q, blk, c0, ncol, op, eng_tt):
                # Cp[blk][:, c0:c0+ncol, 1:52] = Q[., 0, :] op Q[., 1, :]
                dst = bass.AP(tensor=Cp[:, :].tensor,
                              offset=Cp[:, :].offset + blk * NCOL * TP + c0 * TP + 1,
                              ap=[[2 * NCOL * TP, F], [TP, ncol], [1, T]])
                s0 = bass.AP(tensor=Q[:, :].tensor, offset=Q[:, :].offset + c0 * 2 * T,
                             ap=[[NCOL * 2 * T, F], [2 * T, ncol], [1, T]])
                s1 = bass.AP(tensor=Q[:, :].tensor, offset=Q[:, :].offset + c0 * 2 * T + T,
                             ap=[[NCOL * 2 * T, F], [2 * T, ncol], [1, T]])
                eng_tt(dst, s0, s1, op)

            def cp_wrap(blk, c0, ncol):
                dst = bass.AP(tensor=Cp[:, :].tensor,
                              offset=Cp[:, :].offset + blk * NCOL * TP + c0 * TP,
                              ap=[[2 * NCOL * TP, F], [TP, ncol]])
                src = bass.AP(tensor=Cp[:, :].tensor,
                              offset=Cp[:, :].offset + blk * NCOL * TP + c0 * TP + T,
                              ap=[[2 * NCOL * TP, F], [TP, ncol]])
                nc.gpsimd.tensor_copy(dst, src)

            def c_rollsum(blk, dst_blk, c0, ncol, eng_tt):
                dst = bass.AP(tensor=Csb[:, :].tensor,
                              offset=Csb[:, :].offset + dst_blk * BLC + c0 * TP,
                              ap=[[2 * BLC, F], [TP, ncol], [1, T]])
                s0 = bass.AP(tensor=Cp[:, :].tensor,
                             offset=Cp[:, :].offset + blk * NCOL * TP + c0 * TP + 1,
                             ap=[[2 * NCOL * TP, F], [TP, ncol], [1, T]])
                s1 = bass.AP(tensor=Cp[:, :].tensor,
                             offset=Cp[:, :].offset + blk * NCOL * TP + c0 * TP,
                             ap=[[2 * NCOL * TP, F], [TP, ncol], [1, T]])
                eng_tt(dst, s0, s1, ADD)

            def rb_view(RB, s0, nj):
                return bass.AP(tensor=RB[:, :].tensor,
                               offset=RB[:, :].offset + s0 * 2 * T,
                               ap=[[5 * 2 * T, F], [2 * T, nj], [T, 2], [1, T]])

            def c_half_a():
                # cols 0-4 from R slots 0-4 (RB0 psum)
                TTv(q_view(QB, 0, 5), xbc(True, 5), rs_rri(0, 5), MUL)
                TTv(q_view(QA, 0, 5), xbc(False, 5), rs_rri(0, 5), MUL)
                cp_addsub(QB, 1, 0, 5, SUB, TTv)
                cp_addsub(QA, 0, 0, 5, ADD, TTg)
                cp_wrap(1, 0, 5)
                cp_wrap(0, 0, 5)
                c_rollsum(1, 1, 0, 5, TTv)     # Ci
                c_rollsum(0, 0, 0, 5, TTg)     # Cr

            def c_half_b():
                # cols 5-9 from R slots 10-14 (RB2); col 10 from slot 19 (RB3 s4)
                TTv(q_view(QB, 5, 5), xbc(True, 5), rs_rri(10, 5), MUL)
                TTv(q_view(QA, 5, 5), xbc(False, 5), rs_rri(10, 5), MUL)
                TTv(q_view(QB, 10, 1), xbc(True, 1), rb_view(RBs[3], 4, 1), MUL)
                TTv(q_view(QA, 10, 1), xbc(False, 1), rb_view(RBs[3], 4, 1), MUL)
                cp_addsub(QB, 1, 5, 6, SUB, TTv)
                cp_addsub(QA, 0, 5, 6, ADD, TTg)
                cp_wrap(1, 5, 6)
                cp_wrap(0, 5, 6)
                c_rollsum(1, 1, 5, 6, TTv)
                c_rollsum(0, 0, 5, 6, TTg)

            # ---------------- U matmuls ----------------
            def u_group(g):
                UB = pp.tile([F, 5 * 2 * T], FP32, tag="ps", name=f"UB{g}")
                for sl in range(5):
                    slot = g * 5 + sl
                    col = COL_OF_SLOT[slot]
                    conj = CONJ_SLOT[slot]
                    crc = Csb[:, :].offset + col * TP          # Cr col
                    cic = Csb[:, :].offset + BLC + col * TP    # Ci col
                    if not conj:
                        rhs1 = bass.AP(tensor=Csb[:, :].tensor, offset=crc,
                                       ap=[[2 * BLC, F], [BLC, 2], [1, T]])
                        rhs2 = bass.AP(tensor=Csb[:, :].tensor, offset=cic,
                                       ap=[[2 * BLC, F], [1, T]])
                        rhs3 = bass.AP(tensor=Csb[:, :].tensor, offset=crc,
                                       ap=[[2 * BLC, F], [1, T]])
                    else:
                        rhs1 = bass.AP(tensor=Csb[:, :].tensor, offset=cic,
                                       ap=[[2 * BLC, F], [-BLC, 2], [1, T]])
                        rhs2 = bass.AP(tensor=Csb[:, :].tensor, offset=crc,
                                       ap=[[2 * BLC, F], [1, T]])
                        rhs3 = bass.AP(tensor=Csb[:, :].tensor, offset=cic,
                                       ap=[[2 * BLC, F], [1, T]])
                    j = slot
                    out = UB[:, sl * 2 * T:(sl + 1) * 2 * T]
                    outr = UB[:, sl * 2 * T:sl * 2 * T + T]
                    outi = UB[:, sl * 2 * T + T:(sl + 1) * 2 * T]
                    nc.tensor.matmul(out, Msb[:, (3 * j) * F:(3 * j + 1) * F],
                                     rhs1, start=True, stop=False,
                                     skip_group_check=True)
                    nc.tensor.matmul(outr, Msb[:, (3 * j + 1) * F:(3 * j + 2) * F],
                                     rhs2, start=False, stop=True,
                                     skip_group_check=True)
                    nc.tensor.matmul(outi, Msb[:, (3 * j + 2) * F:(3 * j + 3) * F],
                                     rhs3, start=False, stop=True,
                                     skip_group_check=True)
                return UB

            # ---------------- V products + assembly ----------------
            QV = wpool.tile([F, NJ * QJW], BF16, tag="QV")
            Vt = wpool.tile([F, 2 * 2 * VBL], BF16, tag="Vt")  # [Vr|Vi] per group
            Usb = wpool.tile([F, 2 * 5 * 2 * T], BF16, tag="Usb")
            Dsb2 = wpool.tile([F, 105], BF16, tag="Dsb2")
            nc.gpsimd.memset(Dsb2[:, :], 0.0)

            def v_op1(g, UB):
                # op1: [p1|p2] = U[j,c2,t] * Rs[Rr|Ri]   (vector, psum read)
                j0 = g * 5
                dst1 = bass.AP(tensor=QV[:, :].tensor,
                               offset=QV[:, :].offset + j0 * QJW,
                               ap=[[NJ * QJW, F], [QJW, 5], [T, 2], [1, T]])
                u = UB[:, :].rearrange("p (j c t) -> p j c t", j=5, c=2)
                TTv(dst1, u, rs_rri(j0, 5), MUL)

            def u_evict(g, UB):
                pos = {2: 0, 3: 1}[g]
                ACT(Usb[:, pos * 510:(pos + 1) * 510], UB[:, :], CPY)

            def v_op2(g, UB):
                # op2: [p3N|p4] = U * [RiN|Rr]; g<3: gpsimd from Usb, g=3: vector psum
                j0 = g * 5
                dst2 = bass.AP(tensor=QV[:, :].tensor,
                               offset=QV[:, :].offset + j0 * QJW + 2 * T,
                               ap=[[NJ * QJW, F], [QJW, 5], [T, 2], [1, T]])
                rnr = bass.AP(tensor=Rs[:, :].tensor,
                              offset=Rs[:, :].offset + j0 * RJW,
                              ap=[[NJ * RJW, F], [RJW, 5], [T, 2], [1, T]])
                if g in (2, 3):
                    pos = {2: 0, 3: 1}[g]
                    u = bass.AP(tensor=Usb[:, :].tensor,
                                offset=Usb[:, :].offset + pos * 510,
                                ap=[[2 * 510, F], [2 * T, 5], [T, 2], [1, T]])
                    TTg(dst2, u, rnr, MUL)
                else:
                    u = UB[:, :].rearrange("p (j c t) -> p j c t", j=5, c=2)
                    TTv(dst2, u, rnr, MUL)

            def v_assemble(c, nj, eng1, eng2):
                # Vr = p1 - p2 ; Vi = p4 - p3N   (chunk granularity: c in 0..3)
                base = QV[:, :].offset + c * 5 * QJW
                vb = Vt[:, :].offset + (c // 2) * 2 * VBL + (c % 2) * 5 * T

                def qv(off):
                    return bass.AP(tensor=QV[:, :].tensor, offset=base + off,
                                   ap=[[NJ * QJW, F], [QJW, nj], [1, T]])

                def vo(off):
                    return bass.AP(tensor=Vt[:, :].tensor, offset=vb + off,
                                   ap=[[2 * 2 * VBL, F], [T, nj], [1, T]])

                eng1(vo(0), qv(0), qv(T), SUB)
                eng2(vo(VBL), qv(3 * T), qv(2 * T), SUB)

            # ---------------- G matmuls ----------------
            Dacc = pd.tile([F, 2 * 5 * T], FP32, tag="Dacc", name="Dacc")
            DaccB = pd.tile([F, 2 * T], FP32, tag="DaccB", name="DaccB")

            def g_chunk(c, first, last):
                # 3 wide passes into 5-slot DaccA: A=[Vr|Vi], B1=Vi->Dr, B2=Vr->Di
                g = c // 2
                off = Vt[:, :].offset + g * 2 * VBL + (c % 2) * 5 * T
                rhsA = bass.AP(tensor=Vt[:, :].tensor, offset=off,
                               ap=[[2 * 2 * VBL, F], [VBL, 2], [T, 5], [1, T]])
                rhsB1 = bass.AP(tensor=Vt[:, :].tensor, offset=off + VBL,
                                ap=[[2 * 2 * VBL, F], [T, 5], [1, T]])
                rhsB2 = bass.AP(tensor=Vt[:, :].tensor, offset=off,
                                ap=[[2 * 2 * VBL, F], [T, 5], [1, T]])
                outA = Dacc[:, :].rearrange("p (c2 j t) -> p c2 j t", c2=2, j=5)
                outB1 = bass.AP(tensor=Dacc[:, :].tensor, offset=Dacc[:, :].offset,
                                ap=[[2 * 5 * T, F], [T, 5], [1, T]])
                outB2 = bass.AP(tensor=Dacc[:, :].tensor,
                                offset=Dacc[:, :].offset + 5 * T,
                                ap=[[2 * 5 * T, F], [T, 5], [1, T]])
                nc.tensor.matmul(outA, Gc[:, 0:F], rhsA,
                                 start=first, stop=False, skip_group_check=True)
                nc.tensor.matmul(outB1, Gc[:, 2 * F:3 * F], rhsB1,
                                 start=False, stop=last, skip_group_check=True)
                nc.tensor.matmul(outB2, Gc[:, F:2 * F], rhsB2,
                                 start=False, stop=last, skip_group_check=True)

            def g_chunk1_perj():
                # chunk 1 (slots 5-9): per-j passes accumulating into DaccB[F,102]
                outA = DaccB[:, :].rearrange("p (c2 t) -> p c2 t", c2=2)
                outB1 = DaccB[:, 0:T]
                outB2 = DaccB[:, T:2 * T]
                for sl in range(5):
                    off = Vt[:, :].offset + 5 * T + sl * T
                    rhsA = bass.AP(tensor=Vt[:, :].tensor, offset=off,
                                   ap=[[2 * 2 * VBL, F], [VBL, 2], [1, T]])
                    rhsB1 = bass.AP(tensor=Vt[:, :].tensor, offset=off + VBL,
                                    ap=[[2 * 2 * VBL, F], [1, T]])
                    rhsB2 = bass.AP(tensor=Vt[:, :].tensor, offset=off,
                                    ap=[[2 * 2 * VBL, F], [1, T]])
                    nc.tensor.matmul(outA, Gc[:, 0:F], rhsA,
                                     start=(sl == 0), stop=False,
                                     skip_group_check=True)
                    nc.tensor.matmul(outB1, Gc[:, 2 * F:3 * F], rhsB1,
                                     start=False, stop=(sl == 4),
                                     skip_group_check=True)
                    nc.tensor.matmul(outB2, Gc[:, F:2 * F], rhsB2,
                                     start=False, stop=(sl == 4),
                                     skip_group_check=True)

            # ---- pipelined issue: C halves then U/V/G interleaved ----
            c_half_b()
            UB2 = u_group(2)
            UB3 = u_group(3)
            r_evictB(2, RBs[2])
            r_evictB(3, RBs[3])
            c_half_a()
            r_evictB(1, RBs[1])
            r_evictB(0, RBs[0])
            v_op1(2, UB2)
            u_evict(2, UB2)
            v_op2(2, UB2)
            v_op1(3, UB3)
            u_evict(3, UB3)
            v_op2(3, UB3)
            v_assemble(2, 5, TTv, TTg)
            v_assemble(3, 5, TTv, TTg)
            UB0 = u_group(0)
            UB1 = u_group(1)
            g_chunk(2, True, False)
            g_chunk(3, False, False)
            v_op1(0, UB0)
            v_op2(0, UB0)
            v_assemble(0, 5, TTv, TTv)
            g_chunk(0, False, True)
            v_op1(1, UB1)
            v_op2(1, UB1)
            v_assemble(1, 5, TTv, TTv)
            g_chunk1_perj()
            g_chunk1_perj()
            g_chunk1_perj()

            # ---------------- reduce + overlap-add + scale ----------------
            dv = Dacc[:, :].rearrange("p (c j t) -> p c j t", c=2, j=5)
            D4 = wpool.tile([F, 2 * 4 * T], FP32, tag="D4")
            d4v = D4[:, :].rearrange("p (c j t) -> p c j t", c=2, j=4)
            ACT(d4v, dv[:, :, 0:4, :], CPY)
            tE = wpool.tile([F, 2 * 2 * T], FP32, tag="tE")
            tEv = tE[:, :].rearrange("p (c j t) -> p c j t", c=2, j=2)
            TTv(tEv, d4v[:, :, 0:2, :], d4v[:, :, 2:4, :], ADD)
            tF = wpool.tile([F, 2 * T], FP32, tag="tF")
            tFv = tF[:, :].rearrange("p (c t) -> p c t", c=2)
            TTv(tFv, tEv[:, :, 0, :], tEv[:, :, 1, :], ADD)
            tG = wpool.tile([F, 2 * T], FP32, tag="tG")
            tGv = tG[:, :].rearrange("p (c t) -> p c t", c=2)
            TTv(tGv, tFv, dv[:, :, 4, :], ADD)

            # Dsb2: zero-padded slotted layout [pad | c2-plane(52 slots) x 2]
            dbv = bass.AP(tensor=Dsb2[:, :].tensor, offset=Dsb2[:, :].offset + 1,
                          ap=[[105, F], [52, 2], [1, T]])
            TTv(dbv, tGv, DaccB[:, :].rearrange("p (c t) -> p c t", c=2), ADD)

            # Y[tau, c2, tp] = S1 + S2 via selector matmuls into PSUM, x2 edges
            Yp = pp.tile([HOP, 2 * 52], FP32, tag="ps", name="Yp")
            SEL40 = Gc[:, 3 * F:3 * F + HOP]
            SEL0 = Gc[:, 3 * F + HOP:3 * F + 2 * HOP]
            rhs1 = bass.AP(tensor=Dsb2[:, :].tensor, offset=Dsb2[:, :].offset + 1,
                           ap=[[105, F], [52, 2], [1, 52]])
            rhs2 = bass.AP(tensor=Dsb2[:, :].tensor, offset=Dsb2[:, :].offset,
                           ap=[[105, F], [52, 2], [1, 52]])
            rhs3 = bass.AP(tensor=Dsb2[:, :].tensor, offset=Dsb2[:, :].offset + 1,
                           ap=[[105, F], [52, 2], [1, 1]])
            rhs4 = bass.AP(tensor=Dsb2[:, :].tensor, offset=Dsb2[:, :].offset + 51,
                           ap=[[105, F], [52, 2], [1, 1]])
            out3 = bass.AP(tensor=Yp[:, :].tensor, offset=Yp[:, :].offset,
                           ap=[[2 * 52, HOP], [52, 2], [1, 1]])
            out4 = bass.AP(tensor=Yp[:, :].tensor, offset=Yp[:, :].offset + 51,
                           ap=[[2 * 52, HOP], [52, 2], [1, 1]])
            nc.tensor.matmul(Yp[:, :], SEL0, rhs1, start=True, stop=False,
                             skip_group_check=True)
            nc.tensor.matmul(Yp[:, :], SEL40, rhs2, start=False, stop=False,
                             skip_group_check=True)
            nc.tensor.matmul(out3, SEL0, rhs3, start=False, stop=True,
                             skip_group_check=True)
            nc.tensor.matmul(out4, SEL40, rhs4, start=False, stop=True,
                             skip_group_check=True)
            Ysb = wpool.tile([HOP, 2 * 52], FP32, tag="Ysb")
            ACT(Ysb[:, :], Yp[:, :], CPY)
            nc.sync.dma_start(yv[:, :], Ysb[:, :])
    return nc


# ---------------- host side ----------------

def _dft_consts():
    j = np.arange(F)
    W = np.exp(-2j * np.pi * np.outer(j, j) / F)
    G = np.exp(+2j * np.pi * np.outer(j, j) / F) / F
    return W, G


def _host_consts():
    W, G = _dft_consts()
    fr_c = np.concatenate([W.real, W.imag], axis=1).astype(bfloat16)
    cov = np.zeros(L)
    idx = (np.arange(T)[:, None] * HOP + np.arange(F)[None, :]).reshape(-1)
    np.add.at(cov, idx, 1.0)
    cov = np.where(cov > 0, cov, 1.0)
    return fr_c, cov


def _gr_c_for(p):
    _, G = _dft_consts()
    sc = p / 2.0
    sel40 = np.zeros((F, HOP), np.float64)
    sel40[np.arange(HOP) + HOP, np.arange(HOP)] = 1.0
    sel0 = np.zeros((F, HOP), np.float64)
    sel0[np.arange(HOP), np.arange(HOP)] = 1.0
    return np.concatenate([G.real * sc, G.imag * sc, -G.imag * sc, sel40, sel0],
                          axis=1).astype(bfloat16)


def _mmat(w2, n2):
    g = np.arange(F)[None, :]
    f = np.arange(F)[:, None]
    n1 = ((g - f + 20) % F) - 20
    valid = (n1 >= -20) & (n1 <= 19)
    n1c = np.clip(n1 + 20, 0, 39)
    col = w2[:, n2 + 20]
    return np.where(valid, col[n1c], 0.0)     # [f, g]


def _smat_for(n2_list):
    S = np.zeros((NJ, F, F), np.float32)
    g = np.arange(F)
    order = [2, 3, 0, 1]                  # group packing order (matches RPOS)
    for pos, grp in enumerate(order):
        for sl in range(5):
            n2 = n2_list[grp * 5 + sl]
            S[pos * 5 + sl, (g - n2) % F, g] = 1.0
    return np.ascontiguousarray(S.transpose(1, 0, 2).reshape(F, NJ * F)).astype(bfloat16)


def _mst_for(n2_list, w2):
    Ms = np.zeros((NJ, 3, F, F), np.float64)
    for slot, n2 in enumerate(n2_list):
        if not CONJ_SLOT[slot]:
            Mj = _mmat(w2, n2)
            # Ur = Mr@Cr - Mi@Ci ; Ui = Mr@Ci + Mi@Cr
            Ms[slot, 0] = Mj.real            # pair [Cr|Ci]
            Ms[slot, 1] = -Mj.imag           # @Ci -> Ur
            Ms[slot, 2] = Mj.imag            # @Cr -> Ui
        else:
            n = -n2
            Mp = np.roll(_mmat(w2, n2), n, axis=0)   # M'[f'] = M_{-n}[(f'-n)%F]
            # Ur = M'r@Cr + M'i@Ci ; Ui = M'i@Cr - M'r@Ci
            Ms[slot, 0] = Mp.imag            # pair [Ci|Cr]
            Ms[slot, 1] = Mp.real            # @Cr -> Ur
            Ms[slot, 2] = -Mp.real           # @Ci -> Ui
    return np.ascontiguousarray(
        Ms.transpose(2, 0, 1, 3).reshape(F, NJ * 3 * F)).astype(bfloat16)


def _frame(sig):
    idx = np.arange(T)[None, :] * HOP + np.arange(F)[:, None]
    return sig[idx].astype(np.float32)


def make_in_maps(x_real, x_imag, task_info, w_real, w_imag):
    fr_c, cov = _host_consts()
    b, _, m = x_real.shape
    P = np.power(10.0, task_info[:, 0] / 10.0) / m
    w2 = (np.asarray(w_real) + 1j * np.asarray(w_imag)).reshape(40, 40)
    smats = [_smat_for(J_LISTS[h]) for h in range(2)]
    msts = [_mst_for(J_LISTS[h], w2) for h in range(2)]
    grcs = [_gr_c_for(P[bb]) for bb in range(b)]

    in_maps, shards = [], []
    for bb in range(b):
        for mm in range(m):
            fr_ = _frame(x_real[bb, :, mm])
            fi_ = _frame(x_imag[bb, :, mm])
            xfv = np.concatenate([-fi_, fr_, fi_], axis=1).astype(bfloat16)
            for h in range(2):
                in_maps.append({
                    "xf": xfv,
                    "fr_c": fr_c,
                    "gr_c": grcs[bb],
                    "smat": smats[h],
                    "mst": msts[h],
                })
                shards.append((bb, mm, h))
    return in_maps, shards, P, cov


_NC_CACHE = {}


def kernel(x_real, x_imag, task_info, w_real, w_imag, b_real, b_imag):
    x_real = np.asarray(x_real)
    x_imag = np.asarray(x_imag)
    task_info = np.asarray(task_info)
    b, Lx, m = x_real.shape
    assert (b, Lx, m) == (2, L, 2)

    if "nc" not in _NC_CACHE:
        nc_ = build_program(debug=False)
        nc_.compile()
        _NC_CACHE["nc"] = nc_
    nc = _NC_CACHE["nc"]

    in_maps, shards, P, cov = make_in_maps(x_real, x_imag, task_info, w_real, w_imag)
    from concourse.bass_utils import run_bass_kernel_spmd
    res = run_bass_kernel_spmd(nc, in_maps, list(range(8))).results

    x = (x_real + 1j * x_imag).astype(np.complex64)
    out = x.copy()
    bias = complex(np.asarray(b_real)[0], np.asarray(b_imag)[0])
    bias_sig = np.zeros(L, np.complex64)
    bias_sig[np.arange(T) * HOP] = bias
    bias_sig /= cov
    for i, (bb, mm, h) in enumerate(shards):
        yvv = res[i]["yv"]          # [40, 104] = [tau, (yr(52) | yi(52))]
        yr = yvv[:, 0:52].T.ravel()[:L]
        yi = yvv[:, 52:104].T.ravel()[:L]
        out[bb, :, mm] += yr + 1j * yi
    for bb in range(b):
        for mm in range(m):
            out[bb, :, mm] += (P[bb] * bias_sig).astype(np.complex64)
    return out[:, 20:L - 20, :]
